# revision 4
# baseline (speedup 1.0000x reference)
"""LoRA LayerNorm Trainium2 kernel (8-core data-parallel, raw Bass).

out = x_hat * scale + shift, where
  x_hat    = (x - mean) * rsqrt(var + eps)        (LayerNorm over last dim)
  scale[i] = sum_r A_s[i,r] * B_s[r,i] * 2.0      (low-rank diagonal)
  shift[i] = sum_r A_h[i,r] * B_h[r,i] * 2.0

v2: bf16 HBM I/O. The kernel is memory-bound; f32 I/O costs 64 MiB per
core (187 us at the 358 GB/s HBM-per-NC cap). The host casts x to bf16
before upload and upcasts y after download, halving device traffic to
32 MiB (93.7 us floor). End-to-end rel err ~7e-3 vs the 2e-2 gate
(validated in numpy): bf16 quantization of x and y each contribute
~0.4% of local magnitude.

The tiny [N,4] LoRA diagonals are folded on the host (64K FLOPs); the
device receives scale_vec/shift_vec [N] bf16 and x shards [1024, N]
bf16.

Per-core algorithm (rows on partitions, 8 tiles of [128, 8192] bf16,
x six-buffered). Per-tile DMA budget is 11.7 us (2 MiB load + 2 MiB
store); ACT gets no dtype speedup (1 elem/cycle -> 6.9 us per
full-width pass) while DVE runs bf16 tensor_scalar at 4x (2.1 us) and
STT at 2x (4.3 us), so the four full-width passes per tile are split:
  ACT  (iter t): std(t-1) = Sqrt(u/N + eps); sq(t) = accum(Square(x))
        into fp32 [P,1] (bf16 garbage sink); trailing tiny Copy signals
        the READ_ACCUMULATOR retirement (sem acc).
  DVE  (iter t, w = t-1): p1(w): tb = (x + (-mean)) * scale_bc (STT,
        2x); u(w) = sx*nm + sq = N*var; sum(t) = accum(x + 0) via
        tensor_scalar (4x, garbage sink = xb[w], dead after p1);
        nm(t) = -sx/N; rstd(w) = 1/std; p2(w): x = (tb * rstd) +
        shift_bc in-place into the x buffer (STT, 2x, two halves so
        stores can go at half granularity).
  GPSIMD: issues store DMAs (SWDGE) once p2 halves land, so no
        compute queue ever blocks on a p2 wait.
  SYNC: x tile loads only.
  PE (setup): scale/shift rows land in partition 0 of their broadcast
        tiles; PE replicates them across 128 partitions via
        ones-matmuls into PSUM and ACT evacuates (bf16 round-trip is
        exact).
Tile 0's sumsq runs on DVE (STT x*x, sink tb) instead of ACT so the
eight setup evacs don't sit in front of sq(0) during pipeline fill.
Same-engine RAW on [P,1] scalars is closed with self-semaphores
(producer then_inc -> consumer wait_ge), per the engine pipeline's
lack of a write->read interlock.
var = (sum(x^2) - sum(x)^2/N)/N is safe here (x ~ N(0,1), var ~ 1).
"""

import numpy as np
import ml_dtypes
from contextlib import ExitStack

import concourse.bass as bass
from concourse import mybir
from concourse.bass_utils import run_bass_kernel_spmd

F32 = mybir.dt.float32
BF16 = mybir.dt.bfloat16

# Problem geometry (hardcoded; see module docstring)
B_DIM, S_DIM, N = 2, 4096, 8192
RANK = 4
SCALING = 2.0  # alpha / rank = 8 / 4
EPS = 1e-5
NCORES = 8
ROWS = B_DIM * S_DIM // NCORES  # 1024 rows per core
P = 128
NTILES = ROWS // P              # 8
CHUNK = 2048                    # PE->PSUM evac chunk width
HALF = N // 2                   # p2 / tail store granularity
NBUF = 6                        # x tile buffers


def build_nc() -> bass.Bass:
    nc = bass.Bass()

    x = nc.declare_dram_parameter("x_shard", [ROWS, N], BF16, isOutput=False)
    sv = nc.declare_dram_parameter("scale_vec", [N], BF16, isOutput=False)
    hv = nc.declare_dram_parameter("shift_vec", [N], BF16, isOutput=False)
    y = nc.declare_dram_parameter("y_shard", [ROWS, N], BF16, isOutput=True)

    with ExitStack() as ctx:
        ec = ctx.enter_context
        # big tiles (KiB/partition): 6x16(xb) + 16(garb) + 16(tb) + 2x16(bc)
        xb = [ec(nc.sbuf_tensor(f"xb{i}", [P, N], BF16)) for i in range(NBUF)]
        garb = ec(nc.sbuf_tensor("garb", [P, N], BF16))  # ACT Square sink
        tb = ec(nc.sbuf_tensor("tb", [P, N], BF16))      # p1 output
        scale_bc = ec(nc.sbuf_tensor("scale_bc", [P, N], BF16))
        shift_bc = ec(nc.sbuf_tensor("shift_bc", [P, N], BF16))
        # PE broadcast staging (setup only)
        tbp = ec(nc.psum_tensor("tbp", [P, 2 * CHUNK], F32))
        # per-tile stats scalars
        sx_ = [ec(nc.sbuf_tensor(f"sx{i}", [P, 1], F32)) for i in range(NBUF)]
        sq_ = [ec(nc.sbuf_tensor(f"sq{i}", [P, 1], F32)) for i in range(NBUF)]
        u_ = [ec(nc.sbuf_tensor(f"u{i}", [P, 1], F32)) for i in range(2)]
        nm_ = [ec(nc.sbuf_tensor(f"nm{i}", [P, 1], F32)) for i in range(2)]
        std_ = [ec(nc.sbuf_tensor(f"std{i}", [P, 1], F32)) for i in range(2)]
        rstd_ = [ec(nc.sbuf_tensor(f"rstd{i}", [P, 1], F32)) for i in range(2)]
        zt = ec(nc.sbuf_tensor("zt", [P, 1], F32))
        dacc = ec(nc.sbuf_tensor("dacc", [P, 1], F32))
        eps_t = ec(nc.sbuf_tensor("eps_t", [P, 1], F32))
        ones_t = ec(nc.sbuf_tensor("ones_t", [1, P], BF16))

        sems = {}
        for s in (
            "load0", "load1", "load2", "load3", "load4", "load5",
            "store0", "store1", "store2", "store3", "store4", "store5",
            "rows", "pe", "evac", "acc", "vv", "std", "p2", "const",
            "rs", "nmu", "sxs", "sqd",
        ):
            sems[s] = ec(nc.semaphore(s))
        loadS = [sems[f"load{i}"] for i in range(NBUF)]
        storeS = [sems[f"store{i}"] for i in range(NBUF)]

        with nc.Block() as block:

            @block.sync
            def _(sp):
                # scale/shift rows into partition 0 of the bc tiles
                for vec, dst in ((sv, scale_bc), (hv, shift_bc)):
                    sp.dma_start(
                        out=dst[0:1, :],
                        in_=vec[:].rearrange("(u n) -> u n", u=1),
                    ).then_inc(sems["rows"], 16)
                for t in range(NTILES):
                    b = t % NBUF
                    if t >= NBUF:
                        # xb[b] free for reload once tile t-NBUF retired
                        sp.wait_ge(storeS[b], 16 * (t // NBUF))
                    sp.dma_start(
                        out=xb[b][:], in_=x[t * P:(t + 1) * P, :]
                    ).then_inc(loadS[b], 16)

            @block.tensor
            def _(te):
                # replicate scale/shift rows across all 128 partitions:
                # psum_slice[p, f] = ones[p] * row[f]
                te.wait_ge(sems["const"], 3)
                for vi, dst in enumerate((scale_bc, shift_bc)):
                    te.wait_ge(sems["rows"], 16 * (vi + 1))
                    for s in range(16):
                        g = vi * 4 + s // 4
                        if g >= 2 and s % 4 == 0:
                            # psum bank group g%2 free once chunk g-2 evac'd
                            te.wait_ge(sems["evac"], g - 1)
                        off = (g % 2) * CHUNK + (s % 4) * 512
                        nc.tensor.matmul(
                            tbp[:, off:off + 512],
                            ones_t[:],
                            dst[0:1, s * 512:(s + 1) * 512],
                            start=True,
                            stop=True,
                        ).then_inc(sems["pe"], 1)

            @block.scalar
            def _(sc):
                def evac(c):
                    # partition 0 rewritten with identical bytes (exact)
                    dst = (scale_bc, shift_bc)[c // 4]
                    sc.wait_ge(sems["pe"], 4 * (c + 1))
                    sc.activation(
                        out=dst[:, (c % 4) * CHUNK:(c % 4 + 1) * CHUNK],
                        in_=tbp[:, (c % 2) * CHUNK:(c % 2 + 1) * CHUNK],
                        func=mybir.ActivationFunctionType.Copy,
                        bias=0.0,
                    ).then_inc(sems["evac"], 1)

                def sq(t):
                    b = t % NBUF
                    sc.wait_ge(loadS[b], 16 * (t // NBUF + 1))
                    sc.activation(
                        out=garb[:],
                        in_=xb[b][:],
                        func=mybir.ActivationFunctionType.Square,
                        bias=zt[:],
                        accum_out=sq_[b][:],
                    )
                    # accum_out lands via a separate READ_ACCUMULATOR
                    # instruction after the ACTIVATE; signal readiness
                    # from a trailing op so in-order retirement covers it
                    sc.activation(
                        out=dacc[:],
                        in_=zt[:],
                        func=mybir.ActivationFunctionType.Copy,
                        bias=0.0,
                    ).then_inc(sems["acc"], 1)

                def std(t):
                    sc.wait_ge(sems["vv"], t + 1)
                    sc.activation(
                        out=std_[t % 2][:],
                        in_=u_[t % 2][:],
                        func=mybir.ActivationFunctionType.Sqrt,
                        bias=eps_t[:],
                        scale=1.0 / N,
                    ).then_inc(sems["std"], 1)

                sc.wait_ge(sems["const"], 2)
                for c in range(8):
                    evac(c)          # scale_bc then shift_bc (PE runs ahead)
                for t in range(1, NTILES):
                    std(t - 1)
                    sq(t)            # tile 0's sumsq runs on DVE
                std(NTILES - 1)

            @block.vector
            def _(v):
                v.memset(zt[:], 0.0).then_inc(sems["const"], 1)
                v.memset(eps_t[:], EPS).then_inc(sems["const"], 1)
                v.memset(ones_t[:], 1.0).then_inc(sems["const"], 1)

                def p1(w):
                    v.scalar_tensor_tensor(
                        out=tb[:],
                        in0=xb[w % NBUF][:],
                        scalar=nm_[w % 2][:],
                        in1=scale_bc[:],
                        op0=mybir.AluOpType.add,
                        op1=mybir.AluOpType.mult,
                    )

                def p2(w, h):
                    sl = slice(h * HALF, (h + 1) * HALF)
                    v.scalar_tensor_tensor(
                        out=xb[w % NBUF][:, sl],
                        in0=tb[:, sl],
                        scalar=rstd_[w % 2][:],
                        in1=shift_bc[:, sl],
                        op0=mybir.AluOpType.mult,
                        op1=mybir.AluOpType.add,
                    ).then_inc(sems["p2"], 1)

                def summ(t, sink):
                    # 4x-mode single-src pass; accum_out = fp32 row sum
                    b = t % NBUF
                    v.wait_ge(loadS[b], 16 * (t // NBUF + 1))
                    v.tensor_scalar(
                        sink,
                        xb[b][:],
                        0.0,
                        0.0,
                        mybir.AluOpType.add,
                        mybir.AluOpType.add,
                        accum_out=sx_[b][:],
                    ).then_inc(sems["sxs"], 1)

                def nm(t):
                    v.wait_ge(sems["sxs"], t + 1)  # sx committed
                    v.tensor_scalar_mul(
                        nm_[t % 2][:], sx_[t % NBUF][:], -1.0 / N
                    ).then_inc(sems["nmu"], 1)

                def uu(t):
                    # u = sx*nm + sq = sq - sx^2/N  ( = N*var )
                    if t == 0:
                        v.wait_ge(sems["sqd"], 1)      # DVE sumsq(0)
                    else:
                        v.wait_ge(sems["acc"], t)      # ACT sq(t)
                    v.scalar_tensor_tensor(
                        out=u_[t % 2][:],
                        in0=sx_[t % NBUF][:],
                        scalar=nm_[t % 2][:],
                        in1=sq_[t % NBUF][:],
                        op0=mybir.AluOpType.mult,
                        op1=mybir.AluOpType.add,
                    ).then_inc(sems["vv"], 1)

                # ---- tile 0 prologue: both stats on DVE ----
                summ(0, tb[:])
                v.scalar_tensor_tensor(   # sumsq(0): out = (x+0)*x
                    out=tb[:],
                    in0=xb[0][:],
                    scalar=0.0,
                    in1=xb[0][:],
                    op0=mybir.AluOpType.add,
                    op1=mybir.AluOpType.mult,
                    accum_out=sq_[0][:],
                ).then_inc(sems["sqd"], 1)
                nm(0)
                uu(0)

                for t in range(1, NTILES + 1):
                    w = t - 1          # tile being transformed
                    if w == 0:
                        v.wait_ge(sems["evac"], 4)   # scale_bc resident
                    v.wait_ge(sems["nmu"], w + 1)    # nm(w) committed
                    p1(w)
                    if t < NTILES:
                        summ(t, xb[w % NBUF][:])     # xb[w] dead after p1
                        nm(t)
                    v.wait_ge(sems["std"], w + 1)
                    v.reciprocal(rstd_[w % 2][:], std_[w % 2][:]).then_inc(
                        sems["rs"], 1
                    )
                    if w == 0:
                        v.wait_ge(sems["evac"], 8)   # shift_bc resident
                    v.wait_ge(sems["rs"], w + 1)     # rstd(w) committed
                    p2(w, 0)
                    p2(w, 1)
                    if t < NTILES:
                        uu(t)

            @block.gpsimd
            def _(gp):
                for t in range(NTILES):
                    b = t % NBUF
                    if t < NTILES - 2:
                        gp.wait_ge(sems["p2"], 2 * (t + 1))
                        gp.dma_start(
                            out=y[t * P:(t + 1) * P, :], in_=xb[b][:]
                        ).then_inc(storeS[b], 16)
                    else:
                        # tail: store at half-tile granularity
                        for h in range(2):
                            gp.wait_ge(sems["p2"], 2 * t + h + 1)
                            gp.dma_start(
                                out=y[t * P:(t + 1) * P,
                                      h * HALF:(h + 1) * HALF],
                                in_=xb[b][:, h * HALF:(h + 1) * HALF],
                            ).then_inc(storeS[b], 16)

    return nc


def _prep(x, lora_scale_A, lora_scale_B, lora_shift_A, lora_shift_B):
    x = np.asarray(x, dtype=np.float32).reshape(-1, N)
    x_bf = np.ascontiguousarray(x.astype(ml_dtypes.bfloat16))
    scale = np.einsum(
        "nr,rn->n",
        np.asarray(lora_scale_A, np.float32),
        np.asarray(lora_scale_B, np.float32),
    ) * SCALING
    shift = np.einsum(
        "nr,rn->n",
        np.asarray(lora_shift_A, np.float32),
        np.asarray(lora_shift_B, np.float32),
    ) * SCALING
    args = {
        "scale_vec": np.ascontiguousarray(scale.astype(ml_dtypes.bfloat16)),
        "shift_vec": np.ascontiguousarray(shift.astype(ml_dtypes.bfloat16)),
    }
    return [
        {"x_shard": x_bf[i * ROWS:(i + 1) * ROWS], **args}
        for i in range(NCORES)
    ]


def kernel(x, lora_scale_A, lora_scale_B, lora_shift_A, lora_shift_B):
    in_maps = _prep(x, lora_scale_A, lora_scale_B, lora_shift_A, lora_shift_B)
    nc = build_nc()
    res = run_bass_kernel_spmd(nc, in_maps, core_ids=list(range(NCORES)))
    out = np.concatenate(
        [np.asarray(res.results[i]["y_shard"]) for i in range(NCORES)], axis=0
    ).astype(np.float32)
    return out.reshape(B_DIM, S_DIM, N)


if __name__ == "__main__":
    import reference

    inputs = {k: np.asarray(v) for k, v in reference.setup_inputs().items()}
    expected = np.asarray(reference.reference(**inputs))
    actual = kernel(**inputs)
    err = np.abs(actual - expected)
    denom = np.abs(expected).max()
    print("max abs err:", err.max(), "rel:", err.max() / denom)


# revision 8
# speedup vs baseline: 1.5465x; 1.5465x over previous
"""LoRA LayerNorm Trainium2 kernel (8-core data-parallel, raw Bass).

out = x_hat * scale + shift, where
  x_hat    = (x - mean) * rsqrt(var + eps)        (LayerNorm over last dim)
  scale[i] = sum_r A_s[i,r] * B_s[r,i] * 2.0      (low-rank diagonal)
  shift[i] = sum_r A_h[i,r] * B_h[r,i] * 2.0

v3: bf16 HBM I/O (host casts x down / y up; halves device traffic to
32 MiB per core, 93.7 us DMA floor) with an engine split matched to
HW-measured op costs on this DVE firmware:
  - scalar_tensor_tensor and any DVE op with accum_out run at 1x
    (8.6 us full-width) -- unusable.
  - tensor_tensor bf16 runs 2x (4.33 us), tensor_scalar (up to two
    fp32 [P,1] scalars) runs 4x (2.2 us).
  - ACT runs 1 elem/cyc (7.0 us/pass) regardless of dtype, with a
    free fp32 accumulator and per-partition AP scale/bias.

Per-tile (8 tiles of [128, 8192] bf16; 11.7 us DMA, ~13.3 us engines):
  DVE (iter t, w=t-1): xh(w): th = (x + nm[p]) * rstd[p] (TS 4x);
      g(w): th2 = th * scale_bc (TT 2x); p2(w): x = th2 + shift_bc
      in-place (TT 2x); then tile t's share of Sum(x): two pairwise
      fold TTs over x[:, 0:2304] + a width-576 tensor_reduce -> sxd.
  ACT (iter t): sxa(t) = accum(Copy(x[:, 2304:])) (partial row sum);
      sq(t) = accum(Square(x)); then the whole stats chain as tiny
      activations: t1 = sxd + sxa (bias AP), nm = -t1/N (scale),
      m2 = Square(t1 * 1/N), v2 = eps - m2, rstd = Rsqrt(sq * 1/N +
      v2).  Only two cross-engine sems per tile: sxdS (DVE->ACT) and
      statS (ACT->DVE).
  GPSIMD: issues store DMAs (SWDGE) so no compute queue blocks on p2.
  SYNC: loads; PE: setup broadcast of scale/shift across partitions
      via ones-matmuls into PSUM (ACT evacuates scale + 2 shift
      chunks, DVE evacuates the other 2 to shorten the fill).
Tile 0: Sum(x) is folded fully on DVE (no ACT partial) so ACT's
setup evacs and Square(0) dominate its fill slot.
Same-engine RAW on [P,1] scalars is closed with self-semaphores.
var = (sum(x^2) - sum(x)^2/N)/N is safe here (x ~ N(0,1), var ~ 1).
End-to-end rel err ~7.3e-3 vs the 2e-2 gate (numpy-validated).
"""

import numpy as np
import ml_dtypes
from contextlib import ExitStack

import concourse.bass as bass
from concourse import mybir
from concourse.bass_utils import run_bass_kernel_spmd

F32 = mybir.dt.float32
BF16 = mybir.dt.bfloat16

B_DIM, S_DIM, N = 2, 4096, 8192
RANK = 4
SCALING = 2.0  # alpha / rank
EPS = 1e-5
NCORES = 8
ROWS = B_DIM * S_DIM // NCORES  # 1024 rows per core
P = 128
NTILES = ROWS // P              # 8
CHUNK = 2048                    # PE->PSUM evac chunk width
HALF = N // 2
NBUF = 6
FS = 2560                       # DVE fold share of Sum(x)
F1 = FS // 2                    # 1152
F2 = F1 // 2                    # 576
RN = 1.0 / N


def build_nc() -> bass.Bass:
    nc = bass.Bass()

    x = nc.declare_dram_parameter("x_shard", [ROWS, N], BF16, isOutput=False)
    sv = nc.declare_dram_parameter("scale_vec", [N], BF16, isOutput=False)
    hv = nc.declare_dram_parameter("shift_vec", [N], BF16, isOutput=False)
    y = nc.declare_dram_parameter("y_shard", [ROWS, N], BF16, isOutput=True)

    with ExitStack() as ctx:
        ec = ctx.enter_context
        # KiB/partition: 6x16 xb + 16 garb + 16 th + 16 th2 + 32 bc + 12 fold
        xb = [ec(nc.sbuf_tensor(f"xb{i}", [P, N], BF16)) for i in range(NBUF)]
        garb = ec(nc.sbuf_tensor("garb", [P, N], BF16))   # ACT sink
        th = ec(nc.sbuf_tensor("th", [P, N], BF16))       # x_hat
        th2 = ec(nc.sbuf_tensor("th2", [P, N], BF16))     # x_hat*scale
        scale_bc = ec(nc.sbuf_tensor("scale_bc", [P, N], BF16))
        shift_bc = ec(nc.sbuf_tensor("shift_bc", [P, N], BF16))
        foldb = ec(nc.sbuf_tensor("foldb", [P, 6144], BF16))
        tbp = ec(nc.psum_tensor("tbp", [P, 2 * CHUNK], F32))
        sxa_ = [ec(nc.sbuf_tensor(f"sxa{i}", [P, 1], F32)) for i in range(NBUF)]
        sq_ = [ec(nc.sbuf_tensor(f"sq{i}", [P, 1], F32)) for i in range(NBUF)]
        sxd_ = [ec(nc.sbuf_tensor(f"sxd{i}", [P, 1], F32)) for i in range(2)]
        t1_ = [ec(nc.sbuf_tensor(f"t1{i}", [P, 1], F32)) for i in range(2)]
        v1_ = [ec(nc.sbuf_tensor(f"v1{i}", [P, 1], F32)) for i in range(2)]
        v2_ = [ec(nc.sbuf_tensor(f"v2{i}", [P, 1], F32)) for i in range(2)]
        nm_ = [ec(nc.sbuf_tensor(f"nm{i}", [P, 1], F32)) for i in range(2)]
        rstd_ = [ec(nc.sbuf_tensor(f"rstd{i}", [P, 1], F32)) for i in range(2)]
        zt = ec(nc.sbuf_tensor("zt", [P, 1], F32))
        eps_t = ec(nc.sbuf_tensor("eps_t", [P, 1], F32))
        ones_t = ec(nc.sbuf_tensor("ones_t", [1, P], BF16))

        sems = {}
        for s in (
            "load0", "load1", "load2", "load3", "load4", "load5",
            "store0", "store1", "store2", "store3", "store4", "store5",
            "rows", "pe", "evac", "const", "sxdS", "statS", "p2s", "tc",
        ):
            sems[s] = ec(nc.semaphore(s))
        loadS = [sems[f"load{i}"] for i in range(NBUF)]
        storeS = [sems[f"store{i}"] for i in range(NBUF)]

        AOp = mybir.AluOpType
        AF = mybir.ActivationFunctionType
        tc_count = [0]  # ACT tiny-chain self-sem counter

        with nc.Block() as block:

            @block.sync
            def _(sp):
                for vec, dst in ((sv, scale_bc), (hv, shift_bc)):
                    sp.dma_start(
                        out=dst[0:1, :],
                        in_=vec[:].rearrange("(u n) -> u n", u=1),
                    ).then_inc(sems["rows"], 16)
                for t in range(NTILES):
                    b = t % NBUF
                    if t >= NBUF:
                        sp.wait_ge(storeS[b], 16 * (t // NBUF))
                    sp.dma_start(
                        out=xb[b][:], in_=x[t * P:(t + 1) * P, :]
                    ).then_inc(loadS[b], 16)

            @block.tensor
            def _(te):
                te.wait_ge(sems["const"], 3)
                for vi, dst in enumerate((scale_bc, shift_bc)):
                    te.wait_ge(sems["rows"], 16 * (vi + 1))
                    for s in range(16):
                        g = vi * 4 + s // 4
                        if g >= 2 and s % 4 == 0:
                            te.wait_ge(sems["evac"], g - 1)
                        off = (g % 2) * CHUNK + (s % 4) * 512
                        nc.tensor.matmul(
                            tbp[:, off:off + 512],
                            ones_t[:],
                            dst[0:1, s * 512:(s + 1) * 512],
                            start=True,
                            stop=True,
                        ).then_inc(sems["pe"], 1)

            @block.scalar
            def _(sc):
                def tiny(out, in_, func, bias=0.0, scale=1.0):
                    # tiny-chain ops: self-sem before and after so the
                    # next op's read sees the committed [P,1] write
                    ins = sc.activation(
                        out=out, in_=in_, func=func, bias=bias, scale=scale
                    )
                    tc_count[0] += 1
                    ins.then_inc(sems["tc"], 1)
                    sc.wait_ge(sems["tc"], tc_count[0])

                def stats_chain(t):
                    b = t % NBUF
                    p = t % 2
                    sc.wait_ge(sems["sxdS"], t + 1)
                    if t == 0:
                        src = sxd_[0]
                    else:
                        # t1 = sxd + sxa  (bias rides as a [P,1] AP)
                        tiny(t1_[p][:], sxd_[p][:], AF.Copy,
                             bias=sxa_[b][:])
                        src = t1_[p]
                    tiny(nm_[p][:], src[:], AF.Copy, scale=-RN)
                    tiny(v1_[p][:], src[:], AF.Square, scale=RN)
                    tiny(v2_[p][:], v1_[p][:], AF.Copy, bias=eps_t[:],
                         scale=-1.0)
                    sc.activation(
                        out=rstd_[p][:], in_=sq_[b][:], func=AF.Rsqrt,
                        bias=v2_[p][:], scale=RN,
                    ).then_inc(sems["statS"], 1)

                def evac(c, dst, psl, dsl):
                    sc.wait_ge(sems["pe"], 4 * (c + 1))
                    sc.activation(
                        out=dst[:, dsl], in_=tbp[:, psl],
                        func=AF.Copy, bias=0.0,
                    ).then_inc(sems["evac"], 1)

                sc.wait_ge(sems["const"], 2)
                for c in range(4):  # scale_bc
                    evac(c, scale_bc,
                         slice((c % 2) * CHUNK, (c % 2 + 1) * CHUNK),
                         slice(c * CHUNK, (c + 1) * CHUNK))
                # tile 0: Square only (Sum(x) folds fully on DVE)
                sc.wait_ge(loadS[0], 16)
                sc.activation(out=garb[:], in_=xb[0][:], func=AF.Square,
                              bias=zt[:], accum_out=sq_[0][:])
                stats_chain(0)
                # shift chunks c6, c7 (c4, c5 are evacuated by DVE)
                evac(6, shift_bc, slice(0, CHUNK), slice(2 * CHUNK, 3 * CHUNK))
                evac(7, shift_bc, slice(CHUNK, 2 * CHUNK),
                     slice(3 * CHUNK, 4 * CHUNK))
                for t in range(1, NTILES):
                    b = t % NBUF
                    sc.wait_ge(loadS[b], 16 * (t // NBUF + 1))
                    sc.activation(out=garb[:, FS:N], in_=xb[b][:, FS:N],
                                  func=AF.Copy, bias=0.0,
                                  accum_out=sxa_[b][:])
                    sc.activation(out=garb[:], in_=xb[b][:], func=AF.Square,
                                  bias=zt[:], accum_out=sq_[b][:])
                    stats_chain(t)

            @block.vector
            def _(v):
                v.memset(zt[:], 0.0).then_inc(sems["const"], 1)
                v.memset(eps_t[:], EPS).then_inc(sems["const"], 1)
                v.memset(ones_t[:], 1.0).then_inc(sems["const"], 1)

                def fold(t):
                    # tile t's share of Sum(x): x[:, 0:FS] -> sxd_[t%2]
                    bt = t % NBUF
                    v.wait_ge(loadS[bt], 16 * (t // NBUF + 1))
                    v.tensor_tensor(foldb[:, 0:F1], xb[bt][:, 0:F1],
                                    xb[bt][:, F1:FS], AOp.add)
                    v.tensor_tensor(foldb[:, 4096:4096 + F2],
                                    foldb[:, 0:F2], foldb[:, F2:F1],
                                    AOp.add)
                    v.tensor_reduce(sxd_[t % 2][:],
                                    foldb[:, 4096:4096 + F2],
                                    axis=mybir.AxisListType.X,
                                    op=AOp.add).then_inc(sems["sxdS"], 1)

                def transform(w):
                    bw = w % NBUF
                    pw = w % 2
                    v.wait_ge(sems["statS"], w + 1)
                    v.tensor_scalar(th[:], xb[bw][:], nm_[pw][:],
                                    rstd_[pw][:], AOp.add, AOp.mult)
                    if w == 0:
                        v.wait_ge(sems["evac"], 4)   # scale_bc resident
                    v.tensor_tensor(th2[:], th[:], scale_bc[:], AOp.mult)
                    if w == 0:
                        v.wait_ge(sems["evac"], 8)   # shift_bc resident
                    if w < NTILES - 2:
                        v.tensor_tensor(xb[bw][:], th2[:], shift_bc[:],
                                        AOp.add).then_inc(sems["p2s"], 1)
                    else:
                        for h in range(2):
                            sl = slice(h * HALF, (h + 1) * HALF)
                            v.tensor_tensor(xb[bw][:, sl], th2[:, sl],
                                            shift_bc[:, sl],
                                            AOp.add).then_inc(sems["p2s"], 1)

                # iter 0: full-row fold of tile 0 + shift evacs c4, c5
                v.wait_ge(loadS[0], 16)
                v.tensor_tensor(foldb[:, 0:4096], xb[0][:, 0:4096],
                                xb[0][:, 4096:8192], AOp.add)
                v.tensor_tensor(foldb[:, 4096:6144], foldb[:, 0:2048],
                                foldb[:, 2048:4096], AOp.add)
                v.tensor_tensor(foldb[:, 0:1024], foldb[:, 4096:5120],
                                foldb[:, 5120:6144], AOp.add)
                v.tensor_reduce(sxd_[0][:], foldb[:, 0:1024],
                                axis=mybir.AxisListType.X,
                                op=AOp.add).then_inc(sems["sxdS"], 1)
                for ci in range(2):  # shift chunks c4, c5 from PSUM
                    v.wait_ge(sems["pe"], 4 * (5 + ci))
                    v.tensor_copy(
                        shift_bc[:, ci * CHUNK:(ci + 1) * CHUNK],
                        tbp[:, ci * CHUNK:(ci + 1) * CHUNK],
                    ).then_inc(sems["evac"], 1)
                # iter 1: fold(1) only (transform lags stats by 2 tiles)
                fold(1)
                # iters 2..9: [fold(t), transform(t-2)]
                for t in range(2, NTILES + 2):
                    if t < NTILES:
                        fold(t)
                    transform(t - 2)

            @block.gpsimd
            def _(gp):
                np2 = 0
                for t in range(NTILES):
                    b = t % NBUF
                    if t < NTILES - 2:
                        np2 += 1
                        gp.wait_ge(sems["p2s"], np2)
                        gp.dma_start(
                            out=y[t * P:(t + 1) * P, :], in_=xb[b][:]
                        ).then_inc(storeS[b], 16)
                    else:
                        for h in range(2):
                            np2 += 1
                            gp.wait_ge(sems["p2s"], np2)
                            gp.dma_start(
                                out=y[t * P:(t + 1) * P,
                                      h * HALF:(h + 1) * HALF],
                                in_=xb[b][:, h * HALF:(h + 1) * HALF],
                            ).then_inc(storeS[b], 16)

    return nc


def _prep(x, lora_scale_A, lora_scale_B, lora_shift_A, lora_shift_B):
    x = np.asarray(x, dtype=np.float32).reshape(-1, N)
    x_bf = np.ascontiguousarray(x.astype(ml_dtypes.bfloat16))
    scale = np.einsum(
        "nr,rn->n",
        np.asarray(lora_scale_A, np.float32),
        np.asarray(lora_scale_B, np.float32),
    ) * SCALING
    shift = np.einsum(
        "nr,rn->n",
        np.asarray(lora_shift_A, np.float32),
        np.asarray(lora_shift_B, np.float32),
    ) * SCALING
    args = {
        "scale_vec": np.ascontiguousarray(scale.astype(ml_dtypes.bfloat16)),
        "shift_vec": np.ascontiguousarray(shift.astype(ml_dtypes.bfloat16)),
    }
    return [
        {"x_shard": x_bf[i * ROWS:(i + 1) * ROWS], **args}
        for i in range(NCORES)
    ]


def kernel(x, lora_scale_A, lora_scale_B, lora_shift_A, lora_shift_B):
    in_maps = _prep(x, lora_scale_A, lora_scale_B, lora_shift_A, lora_shift_B)
    nc = build_nc()
    res = run_bass_kernel_spmd(nc, in_maps, core_ids=list(range(NCORES)))
    out = np.concatenate(
        [np.asarray(res.results[i]["y_shard"]) for i in range(NCORES)], axis=0
    ).astype(np.float32)
    return out.reshape(B_DIM, S_DIM, N)


if __name__ == "__main__":
    import reference

    inputs = {k: np.asarray(v) for k, v in reference.setup_inputs().items()}
    expected = np.asarray(reference.reference(**inputs))
    actual = kernel(**inputs)
    err = np.abs(actual - expected)
    denom = np.abs(expected).max()
    print("max abs err:", err.max(), "rel:", err.max() / denom)
